# revision 28
# baseline (speedup 1.0000x reference)
"""Single-head causal attention (B=4, T=4096, D=512, H=128) on 8 TRN2 NeuronCores.

Sharding: data-parallel over batch (4 batches x 2 cores). The two cores of a
batch split the 32 query tiles zig-zag style so causal work is balanced
(each core gets one long-context and one short-context tile per pair).
One SPMD program serves both core "types": per-slot k-tile counts are padded
to a shared compile-time schedule, and the causal boundary is applied with
per-core 0/1 mask tiles supplied as input data (built on host, DMA'd).

Per-core device program (all matmuls fp16 inputs -> fp32 PSUM):
  K^T = (wk^T @ x^T), V = x @ wv, Q^T = (wq'^T @ xq^T), wq' = wq/sqrt(H)
  per slot group (4 slots, k-outer): S^T[k,q] = K^T_tile.T @ Q^T block
  P = exp(S^T) (no max subtraction: scores are O(5), fp16 holds exp fine),
  boundary tiles multiplied by 0/1 masks, O[q, 0:129] += P^T.T @ [V | 1]
The ones-column of V carries the softmax denominator through the same PSUM
accumulation; the final division by it (and the +bv bias, which commutes with
the softmax-weighted average) happens on the host after the gather.

Input DMA is spread across the two HWDGE rings (sync + scalar) plus the
SWDGE ring (gpsimd) so transfer fixed-costs overlap and the PE never idles
long enough for HAM to re-throttle the clock.
"""

import numpy as np
import ml_dtypes

B, T, D, H = 4, 4096, 512, 128
P = 128          # partitions / tile edge
DO = D // P      # contraction chunks (4)
NT = T // P      # k tiles per batch (32)
NS = 16          # query-tile slots per core
TQ = NS * P      # queries per core (2048)
KC = [32 - 2 * s for s in range(NS)]   # k-tiles processed per slot (desc)
VW = 132         # per-k-tile V row pitch: [V(128) | ones | pad] (4B aligned)

_f16 = np.float16

_CACHE = {}


_QTILE_A = [31, 28, 27, 24, 23, 20, 19, 16, 15, 12, 11, 8, 7, 4, 3, 0]
_QTILE_B = [30, 29, 26, 25, 22, 21, 18, 17, 14, 13, 10, 9, 6, 5, 2, 1]


def _slot_qtile(core_type: int):
    """Global q-tile index handled by each slot, for core type 0/1.

    The {4k+3,4k} / {4k+2,4k+1} split gives both core types exactly 264 real
    k-tiles under the shared padded schedule KC (each type pads 8 slots by
    one tile), so the end-of-kernel all-core barrier sees balanced work.
    """
    return list(_QTILE_A if core_type == 0 else _QTILE_B)


def _build_program():
    import concourse.tile as tile
    from concourse import bacc, mybir
    from concourse.bass import ts, ds

    f16 = mybir.dt.float16
    f32 = mybir.dt.float32
    Exp = mybir.ActivationFunctionType.Exp

    nc = bacc.Bacc("TRN2", target_bir_lowering=False, debug=False, num_devices=8)

    xT_d = nc.dram_tensor("xT", [P, DO, T], f16, kind="ExternalInput").ap()
    xqT_d = nc.dram_tensor("xqT", [P, DO, TQ], f16, kind="ExternalInput").ap()
    wall_d = nc.dram_tensor("wall", [P, 3, DO, P], f16, kind="ExternalInput").ap()
    ball_d = nc.dram_tensor("ball", [P, 2], f32, kind="ExternalInput").ap()
    msk_d = nc.dram_tensor("msk", [P, NS * 2 * P], f16, kind="ExternalInput").ap()
    out_d = nc.dram_tensor("out", [NS, P, 129], f32, kind="ExternalOutput").ap()

    NSTRIP = T // 512          # 8 key strips
    NQSTRIP = TQ // 512        # 4 query strips (one per slot group)

    with tile.TileContext(nc) as tc:
        # Warm-up matmuls on zeroed scratch: no data dependencies, so they
        # run during the input-DMA lead-in and lift the PE HAM clock gate to
        # 2.4 GHz before the first real matmul arrives.  The scratch pool
        # stays open for the whole kernel so no later tile aliases its SBUF
        # (a WAR dep there would stall the first input DMA behind the warm
        # matmuls).  Only the PSUM bank is released.
        with tc.tile_pool(name="const", bufs=1) as cpool, \
             tc.tile_pool(name="warm", bufs=1) as wmp, \
             tc.tile_pool(name="data", bufs=1) as dpool:
            scr = wmp.tile([P, 512], f16)
            nc.vector.memset(scr[:], 0.0)
            with tc.tile_pool(name="warmps", bufs=1, space="PSUM") as wps:
                wps_t = wps.tile([P, 512], f32)
                for _ in range(8):
                    nc.tensor.matmul(wps_t, scr[:, 0:128], scr[:],
                                     start=True, stop=True)
            wall_sb = cpool.tile([P, 3, DO, P], f16)
            ball_sb = cpool.tile([P, 2], f32)
            msk_sb = cpool.tile([P, NS * 2 * P], f16)

            # per-strip tiles so dependencies stay fine-grained: attention on
            # early k-tiles runs while later x strips are still in flight.
            xq_t = [dpool.tile([P, DO, 512], f16, name=f"xq_{i}")
                    for i in range(NQSTRIP)]
            xt_t = [dpool.tile([P, DO, 512], f16, name=f"xt_{i}")
                    for i in range(NSTRIP)]
            qt_t = [dpool.tile([P, 512], f16, name=f"qt_{i}")
                    for i in range(NQSTRIP)]
            kt_t = [dpool.tile([P, 512], f16, name=f"kt_{i}")
                    for i in range(NSTRIP)]
            v4_t = [dpool.tile([P, 4, VW], f16, name=f"v_{i}")
                    for i in range(NSTRIP)]

            # All input DMA shares one ~350 GB/s HBM pipe and rings are
            # drained round-robin, so a SINGLE ring in strict priority order
            # gives exact control: weights first (gate the first LDWEIGHTS),
            # then strip 0 of Q and K/V, then k strips in consumption order.
            # Late-needed data (xq1-3 for phase 2, masks) goes last so it
            # never competes with the critical strips.
            nc.gpsimd.dma_start(ball_sb[:], ball_d)   # tiny; off the hot ring
            nc.sync.dma_start(wall_sb[:], wall_d)
            nc.sync.dma_start(xt_t[0][:, 0:2], xT_d[:, 0:2, ts(0, 512)])
            nc.sync.dma_start(xt_t[0][:, 2:4], xT_d[:, 2:4, ts(0, 512)])
            nc.sync.dma_start(xq_t[0][:], xqT_d[:, :, ts(0, 512)])
            nc.sync.dma_start(xt_t[1][:], xT_d[:, :, ts(1, 512)])
            nc.sync.dma_start(xt_t[2][:], xT_d[:, :, ts(2, 512)])
            nc.sync.dma_start(xq_t[1][:], xqT_d[:, :, ts(1, 512)])
            nc.sync.dma_start(xt_t[3][:], xT_d[:, :, ts(3, 512)])
            nc.sync.dma_start(xt_t[4][:], xT_d[:, :, ts(4, 512)])
            nc.sync.dma_start(xq_t[2][:], xqT_d[:, :, ts(2, 512)])
            nc.sync.dma_start(xq_t[3][:], xqT_d[:, :, ts(3, 512)])
            nc.sync.dma_start(xt_t[5][:], xT_d[:, :, ts(5, 512)])
            nc.sync.dma_start(xt_t[6][:], xT_d[:, :, ts(6, 512)])
            nc.sync.dma_start(msk_sb[:], msk_d)
            nc.sync.dma_start(xt_t[7][:], xT_d[:, :, ts(7, 512)])
            for st in range(NSTRIP):                 # softmax-denominator ones
                nc.gpsimd.memset(v4_t[st][:, :, 128:129], 1.0)

            def grp_w(g, u):
                return sum(1 for s in range(4 * g, 4 * g + 4) if KC[s] > u)

            # prefetch schedule: strip -> [(g, u0) score pairs for groups 1-3]
            # computed during phase 1 and stored as exp'd fp16 tiles so phase 2
            # is a pure PV sweep.  Each entry needs k-tiles u0, u0+1 (<= strip)
            # and qt_t[g] (projected at strips 1, 3, 4).
            PREF = {2: [(1, 0), (1, 2), (1, 4), (1, 6)],
                    3: [(1, 8), (1, 10), (1, 12), (1, 14)],
                    4: [(1, 16), (1, 18), (2, 0), (2, 2)],
                    5: [(1, 20), (1, 22), (2, 4), (2, 6)],
                    6: [(2, 8), (2, 10)],
                    7: [(2, 12), (2, 14)]}
            p_st = {}      # (g, u0) -> stored exp'd score pair tile

            with tc.tile_pool(name="sb_w", bufs=6) as wpool, \
                 tc.tile_pool(name="sb_p", bufs=1) as ppool, \
                 tc.tile_pool(name="sb_f", bufs=10) as fpool:

                def finalize(s, o_acc):
                    o_sb = fpool.tile([P, 129], f32, tag="osb", name=f"osb_{s}")
                    nc.vector.tensor_copy(o_sb, o_acc[:, 0:129])
                    nc.sync.dma_start(out_d[s], o_sb)

                def boundary_masks(p_ap_fn, u, slots, w):
                    for ci, s in enumerate(slots[:w]):
                        if u >= KC[s] - 2:
                            i = u - (KC[s] - 2)
                            nc.vector.tensor_mul(
                                p_ap_fn(ci), p_ap_fn(ci),
                                msk_sb[:, ds((2 * s + i) * P, P)])

                def proj_q(pp, st):
                    ps = pp.tile([P, 512], f32, tag="proj", name=f"psq_{st}")
                    for o in range(DO):
                        nc.tensor.matmul(ps, wall_sb[:, 0, o], xq_t[st][:, o],
                                         start=(o == 0), stop=(o == DO - 1))
                    nc.vector.tensor_scalar_add(qt_t[st][:], ps, ball_sb[:, 0:1])

                def proj_kv(pp, st):
                    ps = pp.tile([P, 512], f32, tag="proj", name=f"psk_{st}")
                    for o in range(DO):
                        nc.tensor.matmul(ps, wall_sb[:, 1, o], xt_t[st][:, o],
                                         start=(o == 0), stop=(o == DO - 1))
                    nc.vector.tensor_scalar_add(kt_t[st][:], ps, ball_sb[:, 1:2])
                    ps_v = pp.tile([P, 4, P], f32, tag="proj", name=f"psv_{st}")
                    for j in range(4):
                        for o in range(DO):
                            nc.tensor.matmul(ps_v[:, j], xt_t[st][:, o, ts(j, P)],
                                             wall_sb[:, 2, o],
                                             start=(o == 0), stop=(o == DO - 1))
                    nc.vector.tensor_copy(v4_t[st][:, :, 0:128], ps_v)

                def score_pair(ps0, g, u0, store):
                    """S^T for k-tiles u0,u0+1 of group g -> exp'd fp16 tile."""
                    slots = list(range(4 * g, 4 * g + 4))
                    w0 = grp_w(g, u0)
                    s_sp = ps0.tile([P, 2, 512], f32, tag="s0",
                                    name=f"s_{g}_{u0}")
                    for j in range(2):
                        u = u0 + j
                        wj = grp_w(g, u)
                        nc.tensor.matmul(s_sp[:, j, 0:wj * P],
                                         kt_t[u // 4][:, ts(u % 4, P)],
                                         qt_t[g][:, 0:wj * P],
                                         start=True, stop=True)
                    if store:
                        p_sb = ppool.tile([P, 2, 512], f16, name=f"p_{g}_{u0}")
                        p_st[(g, u0)] = p_sb
                    else:
                        p_sb = wpool.tile([P, 2, 512], f16, tag="ptile0",
                                          name=f"p_{g}_{u0}")
                    nc.scalar.activation(p_sb[:, :, 0:w0 * P],
                                         s_sp[:, :, 0:w0 * P], Exp)
                    for j in range(2):
                        boundary_masks(
                            lambda ci, j=j: p_sb[:, j, ts(ci, P)],
                            u0 + j, slots, grp_w(g, u0 + j))
                    return p_sb

                # ---- phase 1: projections, group-0 attention, and S+exp
                # prefetch for groups 1-3 interleaved over the k strips ----
                slots0 = [0, 1, 2, 3]
                first0 = {0, 2}
                with tc.tile_pool(name="ps_o0", bufs=1, space="PSUM") as po0, \
                     tc.tile_pool(name="pproj", bufs=2, space="PSUM") as pp, \
                     tc.tile_pool(name="ps_s0", bufs=2, space="PSUM") as ps0:
                    ob0 = {i: po0.tile([P, 2, 129], f32,
                                       name=f"o_bank_0_{i}") for i in range(2)}
                    o_ps0 = {s: ob0[ci // 2][:, ci % 2]
                             for ci, s in enumerate(slots0)}
                    def g0_pairs(stp):
                        for up in (2 * stp, 2 * stp + 1):
                            u0 = 2 * up
                            w = grp_w(0, u0)
                            p_sb = score_pair(ps0, 0, u0, store=False)
                            for j in range(2):
                                u = u0 + j
                                for ci, s in enumerate(slots0[:w]):
                                    nc.tensor.matmul(
                                        o_ps0[s], p_sb[:, j, ts(ci, P)],
                                        v4_t[u // 4][:, u % 4, 0:129],
                                        start=(u == 0 and s in first0),
                                        stop=(u == KC[s] - 1),
                                        skip_group_check=True)
                                    if u == KC[s] - 1:
                                        finalize(s, o_ps0[s])

                    # attention pairs run one strip behind the projections so
                    # the in-order Tensor queue never head-blocks on a strip
                    # (or q-block) whose DMA is still in flight; pairs go
                    # first within the iteration since their data is already
                    # resident while strip st's DMA may still be landing.
                    QP = {0: 0, 2: 1, 4: 2, 5: 3}     # strip -> proj_q(g)
                    for st in range(NSTRIP):
                        if st >= 1:
                            g0_pairs(st - 1)
                        proj_kv(pp, st)
                        if st in QP:
                            proj_q(pp, QP[st])
                        for g, u0 in PREF.get(st, ()):
                            score_pair(ps0, g, u0, store=True)
                    g0_pairs(NSTRIP - 1)

                # ---- phase 2: pure PV sweep over the stored exp'd scores,
                # merged u-major across groups 1-3 so the narrow tails of the
                # groups overlap.  Six independent O banks (PSUM freed by the
                # phase-1 pools) so groups never wait on one another's
                # finalize.  O accumulators pack two slots per PSUM bank,
                # relying on per-element has_written: the bank's first PV
                # (slot A, u=0, start=True) clears the bank; slot B's u=0 PV
                # uses start=False and overwrites its still-unwritten
                # elements.
                with tc.tile_pool(name="ps_o2", bufs=1, space="PSUM") as po2, \
                     tc.tile_pool(name="ps_s2", bufs=1, space="PSUM") as ps2:
                    o_ps = {}
                    first_in_bank = set()
                    for g in range(1, 4):
                        slots = list(range(4 * g, 4 * g + 4))
                        ob = {i: po2.tile([P, 2, 129], f32, name=f"o2_{g}_{i}")
                              for i in range(2)}
                        for ci, s in enumerate(slots):
                            o_ps[s] = ob[ci // 2][:, ci % 2]
                        first_in_bank |= {slots[0], slots[2]}
                    for u0 in range(0, KC[4], 2):
                        # scores not prefetched in phase 1 are computed here,
                        # hidden behind the other groups' PV streams (Scalar
                        # has plenty of slack in this phase).
                        for g in (2, 3):
                            if grp_w(g, u0) and (g, u0) not in p_st:
                                p_st[(g, u0)] = score_pair(ps2, g, u0,
                                                           store=False)
                        for jj in range(2):
                            u = u0 + jj
                            for g in range(1, 4):
                                wj = grp_w(g, u)
                                if wj == 0:
                                    continue
                                slots = list(range(4 * g, 4 * g + 4))
                                p_sb = p_st[(g, u0)]
                                for ci, s in enumerate(slots[:wj]):
                                    nc.tensor.matmul(
                                        o_ps[s], p_sb[:, jj, ts(ci, P)],
                                        v4_t[u // 4][:, u % 4, 0:129],
                                        start=(u == 0 and s in first_in_bank),
                                        stop=(u == KC[s] - 1),
                                        skip_group_check=True)
                                    if u == KC[s] - 1:
                                        finalize(s, o_ps[s])

    nc.compile()
    return nc


def _prep_core(core, x, wq, bq, wk, bk, wv, bv):
    b, ct = core // 2, core % 2
    qtiles = _slot_qtile(ct)
    scale = np.float32(1.0 / np.sqrt(H))

    def dchunk(a):  # [D, N] -> [P, DO, N] with d = o*P + p
        return np.ascontiguousarray(
            a.reshape(DO, P, -1).transpose(1, 0, 2)).astype(_f16)

    xT = x[b].T.astype(np.float32)                      # [D, T]
    qrows = np.concatenate([np.arange(j * P, (j + 1) * P) for j in qtiles])
    xqT = np.ascontiguousarray(xT[:, qrows])            # [D, TQ]

    wall = np.stack([dchunk(wq * scale), dchunk(wk), dchunk(wv)], axis=1)
    ball = np.stack([(bq * scale).astype(np.float32),
                     bk.astype(np.float32)], axis=1)

    # boundary masks: per slot, blocks for u = KC-2 and KC-1 in S^T [k, q]
    # orientation: ones below the causal boundary, triu on the diagonal tile,
    # zeros for the padded tile beyond it.
    triu = np.triu(np.ones((P, P), dtype=_f16))
    msk = np.zeros((P, NS * 2 * P), dtype=_f16)
    for s in range(NS):
        j = qtiles[s]
        for i in range(2):
            u = KC[s] - 2 + i
            blk = 2 * s + i
            if u < j:
                msk[:, blk * P:(blk + 1) * P] = 1.0
            elif u == j:
                msk[:, blk * P:(blk + 1) * P] = triu

    return {
        "xT": dchunk(xT),
        "xqT": dchunk(xqT),
        "wall": wall,
        "ball": ball,
        "msk": msk,
    }


def _fallback(x, mask, wq, bq, wk, bk, wv, bv):
    """Exact numpy path for inputs the specialized kernel doesn't cover."""
    out = np.empty((x.shape[0], x.shape[1], wq.shape[1]), dtype=np.float32)
    scale = np.float32(1.0 / np.sqrt(wq.shape[1]))
    for b in range(x.shape[0]):
        q = x[b] @ wq + bq
        k = x[b] @ wk + bk
        v = x[b] @ wv + bv
        s = (q @ k.T) * scale
        s = np.where(mask == 0, np.float32(-1e30), s)
        s -= s.max(axis=-1, keepdims=True)
        p = np.exp(s)
        p /= p.sum(axis=-1, keepdims=True)
        out[b] = p @ v
    return out


def kernel(**inputs):
    x = np.asarray(inputs["x"], dtype=np.float32)
    mask = np.asarray(inputs["mask"])
    wq = np.asarray(inputs["wq"], dtype=np.float32)
    bq = np.asarray(inputs["bq"], dtype=np.float32)
    wk = np.asarray(inputs["wk"], dtype=np.float32)
    bk = np.asarray(inputs["bk"], dtype=np.float32)
    wv = np.asarray(inputs["wv"], dtype=np.float32)
    bv = np.asarray(inputs["bv"], dtype=np.float32)

    causal = (x.shape == (B, T, D) and wq.shape == (D, H)
              and np.array_equal(mask, np.tril(np.ones((T, T), mask.dtype))))
    if not causal:
        return _fallback(x, mask, wq, bq, wk, bk, wv, bv)

    if "nc" not in _CACHE:
        _CACHE["nc"] = _build_program()
    nc = _CACHE["nc"]

    from concourse import bass_utils
    in_maps = [_prep_core(c, x, wq, bq, wk, bk, wv, bv) for c in range(8)]
    res = bass_utils.run_bass_kernel_spmd(nc, in_maps, core_ids=list(range(8)),
                                          **_CACHE.get("run_kwargs", {}))
    _CACHE["last_result"] = res

    out = np.empty((B, T, H), dtype=np.float32)
    bvf = bv.astype(np.float32)
    for c in range(8):
        b, ct = c // 2, c % 2
        qtiles = _slot_qtile(ct)
        oc = res.results[c]["out"]          # [NS, P, 129]
        for s, j in enumerate(qtiles):
            out[b, j * P:(j + 1) * P, :] = (
                oc[s, :, 0:128] / oc[s, :, 128:129] + bvf)
    return out


# revision 29
# speedup vs baseline: 1.1531x; 1.1531x over previous
"""Single-head causal attention (B=4, T=4096, D=512, H=128) on 8 TRN2 NeuronCores.

Sharding: data-parallel over batch (4 batches x 2 cores). The two cores of a
batch split the 32 query tiles zig-zag style so causal work is balanced
(each core gets one long-context and one short-context tile per pair).
One SPMD program serves both core "types": per-slot k-tile counts are padded
to a shared compile-time schedule, and the causal boundary is applied with
per-core 0/1 mask tiles supplied as input data (built on host, DMA'd).

Per-core device program (all matmuls fp16 inputs -> fp32 PSUM):
  K^T = (wk^T @ x^T), V = x @ wv, Q^T = (wq'^T @ xq^T), wq' = wq/sqrt(H)
  per slot group (4 slots, k-outer): S^T[k,q] = K^T_tile.T @ Q^T block
  P = exp(S^T) (no max subtraction: scores are O(5), fp16 holds exp fine),
  boundary tiles multiplied by 0/1 masks, O[q, 0:129] += P^T.T @ [V | 1]
The ones-column of V carries the softmax denominator through the same PSUM
accumulation; the final division by it (and the +bv bias, which commutes with
the softmax-weighted average) happens on the host after the gather.

Input DMA is spread across the two HWDGE rings (sync + scalar) plus the
SWDGE ring (gpsimd) so transfer fixed-costs overlap and the PE never idles
long enough for HAM to re-throttle the clock.
"""

import numpy as np
import ml_dtypes

B, T, D, H = 4, 4096, 512, 128
P = 128          # partitions / tile edge
DO = D // P      # contraction chunks (4)
NT = T // P      # k tiles per batch (32)
NS = 16          # query-tile slots per core
TQ = NS * P      # queries per core (2048)
KC = [32 - 2 * s for s in range(NS)]   # k-tiles processed per slot (desc)
VW = 132         # per-k-tile V row pitch: [V(128) | ones | pad] (4B aligned)

_f16 = np.float16

_CACHE = {}


_QTILE_A = [31, 28, 27, 24, 23, 20, 19, 16, 15, 12, 11, 8, 7, 4, 3, 0]
_QTILE_B = [30, 29, 26, 25, 22, 21, 18, 17, 14, 13, 10, 9, 6, 5, 2, 1]


def _slot_qtile(core_type: int):
    """Global q-tile index handled by each slot, for core type 0/1.

    The {4k+3,4k} / {4k+2,4k+1} split gives both core types exactly 264 real
    k-tiles under the shared padded schedule KC (each type pads 8 slots by
    one tile), so the end-of-kernel all-core barrier sees balanced work.
    """
    return list(_QTILE_A if core_type == 0 else _QTILE_B)


def _build_program():
    import concourse.tile as tile
    from concourse import bacc, mybir
    from concourse.bass import ts, ds

    f16 = mybir.dt.float16
    f32 = mybir.dt.float32
    Exp = mybir.ActivationFunctionType.Exp

    nc = bacc.Bacc("TRN2", target_bir_lowering=False, debug=False, num_devices=8)

    xT_d = nc.dram_tensor("xT", [P, DO, T], f16, kind="ExternalInput").ap()
    xqT_d = nc.dram_tensor("xqT", [P, DO, TQ], f16, kind="ExternalInput").ap()
    wall_d = nc.dram_tensor("wall", [P, 3, DO, P], f16, kind="ExternalInput").ap()
    ball_d = nc.dram_tensor("ball", [P, 2], f32, kind="ExternalInput").ap()
    msk_d = nc.dram_tensor("msk", [P, NS * 2 * P], f16, kind="ExternalInput").ap()
    out_d = nc.dram_tensor("out", [NS, P, 129], f32, kind="ExternalOutput").ap()

    NSTRIP = T // 512          # 8 key strips
    NQSTRIP = TQ // 512        # 4 query strips (one per slot group)

    with tile.TileContext(nc) as tc:
        # Warm-up matmuls on zeroed scratch: no data dependencies, so they
        # run during the input-DMA lead-in and lift the PE HAM clock gate to
        # 2.4 GHz before the first real matmul arrives.  The scratch pool
        # stays open for the whole kernel so no later tile aliases its SBUF
        # (a WAR dep there would stall the first input DMA behind the warm
        # matmuls).  Only the PSUM bank is released.
        with tc.tile_pool(name="const", bufs=1) as cpool, \
             tc.tile_pool(name="warm", bufs=1) as wmp, \
             tc.tile_pool(name="data", bufs=1) as dpool:
            scr = wmp.tile([P, 512], f16)
            nc.vector.memset(scr[:], 0.0)
            with tc.tile_pool(name="warmps", bufs=1, space="PSUM") as wps:
                wps_t = wps.tile([P, 512], f32)
                for _ in range(8):
                    nc.tensor.matmul(wps_t, scr[:, 0:128], scr[:],
                                     start=True, stop=True)
            wall_sb = cpool.tile([P, 3, DO, P], f16)
            ball_sb = cpool.tile([P, 2], f32)
            msk_sb = cpool.tile([P, NS * 2 * P], f16)

            # per-strip tiles so dependencies stay fine-grained: attention on
            # early k-tiles runs while later x strips are still in flight.
            xq_t = [dpool.tile([P, DO, 512], f16, name=f"xq_{i}")
                    for i in range(NQSTRIP)]
            xt_t = [dpool.tile([P, DO, 512], f16, name=f"xt_{i}")
                    for i in range(NSTRIP)]
            qt_t = [dpool.tile([P, 512], f16, name=f"qt_{i}")
                    for i in range(NQSTRIP)]
            kt_t = [dpool.tile([P, 512], f16, name=f"kt_{i}")
                    for i in range(NSTRIP)]
            v4_t = [dpool.tile([P, 4, VW], f16, name=f"v_{i}")
                    for i in range(NSTRIP)]

            # All input DMA shares one ~350 GB/s HBM pipe and rings are
            # drained round-robin, so a SINGLE ring in strict priority order
            # gives exact control: weights first (gate the first LDWEIGHTS),
            # then strip 0 of Q and K/V, then k strips in consumption order.
            # Late-needed data (xq1-3 for phase 2, masks) goes last so it
            # never competes with the critical strips.
            nc.gpsimd.dma_start(ball_sb[:], ball_d)   # tiny; off the hot ring
            nc.sync.dma_start(wall_sb[:], wall_d)
            nc.sync.dma_start(xt_t[0][:, 0:2], xT_d[:, 0:2, ts(0, 512)])
            nc.sync.dma_start(xt_t[0][:, 2:4], xT_d[:, 2:4, ts(0, 512)])
            nc.sync.dma_start(xq_t[0][:], xqT_d[:, :, ts(0, 512)])
            nc.sync.dma_start(xt_t[1][:], xT_d[:, :, ts(1, 512)])
            nc.sync.dma_start(xt_t[2][:], xT_d[:, :, ts(2, 512)])
            nc.sync.dma_start(xq_t[1][:], xqT_d[:, :, ts(1, 512)])
            nc.sync.dma_start(xt_t[3][:], xT_d[:, :, ts(3, 512)])
            nc.sync.dma_start(xt_t[4][:], xT_d[:, :, ts(4, 512)])
            nc.sync.dma_start(xq_t[2][:], xqT_d[:, :, ts(2, 512)])
            nc.sync.dma_start(xq_t[3][:], xqT_d[:, :, ts(3, 512)])
            nc.sync.dma_start(xt_t[5][:], xT_d[:, :, ts(5, 512)])
            nc.sync.dma_start(xt_t[6][:], xT_d[:, :, ts(6, 512)])
            nc.sync.dma_start(msk_sb[:], msk_d)
            nc.sync.dma_start(xt_t[7][:], xT_d[:, :, ts(7, 512)])
            for st in range(NSTRIP):                 # softmax-denominator ones
                nc.gpsimd.memset(v4_t[st][:, :, 128:129], 1.0)

            def grp_w(g, u):
                return sum(1 for s in range(4 * g, 4 * g + 4) if KC[s] > u)

            # prefetch schedule: strip -> [(g, u0) score pairs for groups 1-3]
            # computed during phase 1 and stored as exp'd fp16 tiles so phase 2
            # is a pure PV sweep.  Each entry needs k-tiles u0, u0+1 (<= strip)
            # and qt_t[g] (projected at strips 1, 3, 4).
            PREF = {2: [(1, 0), (1, 2), (1, 4), (1, 6)],
                    3: [(1, 8), (1, 10), (1, 12), (1, 14)],
                    4: [(1, 16), (1, 18), (2, 0), (2, 2)],
                    5: [(1, 20), (1, 22), (2, 4)],
                    6: [(2, 6), (2, 8), (2, 10)],
                    7: [(2, 12), (2, 14)]}
            p_st = {}      # (g, u0) -> stored exp'd score pair tile

            with tc.tile_pool(name="sb_w", bufs=6) as wpool, \
                 tc.tile_pool(name="sb_p", bufs=1) as ppool, \
                 tc.tile_pool(name="sb_f", bufs=10) as fpool:

                def finalize(s, o_acc):
                    o_sb = fpool.tile([P, 129], f32, tag="osb", name=f"osb_{s}")
                    nc.vector.tensor_copy(o_sb, o_acc[:, 0:129])
                    nc.sync.dma_start(out_d[s], o_sb)

                def boundary_masks(p_ap_fn, u, slots, w):
                    for ci, s in enumerate(slots[:w]):
                        if u >= KC[s] - 2:
                            i = u - (KC[s] - 2)
                            nc.vector.tensor_mul(
                                p_ap_fn(ci), p_ap_fn(ci),
                                msk_sb[:, ds((2 * s + i) * P, P)])

                def proj_q(pp, st):
                    ps = pp.tile([P, 512], f32, tag="proj", name=f"psq_{st}")
                    for o in range(DO):
                        nc.tensor.matmul(ps, wall_sb[:, 0, o], xq_t[st][:, o],
                                         start=(o == 0), stop=(o == DO - 1))
                    nc.vector.tensor_scalar_add(qt_t[st][:], ps, ball_sb[:, 0:1])

                def proj_kv(pp, st):
                    ps = pp.tile([P, 512], f32, tag="proj", name=f"psk_{st}")
                    for o in range(DO):
                        nc.tensor.matmul(ps, wall_sb[:, 1, o], xt_t[st][:, o],
                                         start=(o == 0), stop=(o == DO - 1))
                    nc.vector.tensor_scalar_add(kt_t[st][:], ps, ball_sb[:, 1:2])
                    ps_v = pp.tile([P, 4, P], f32, tag="proj", name=f"psv_{st}")
                    for j in range(4):
                        for o in range(DO):
                            nc.tensor.matmul(ps_v[:, j], xt_t[st][:, o, ts(j, P)],
                                             wall_sb[:, 2, o],
                                             start=(o == 0), stop=(o == DO - 1))
                    nc.vector.tensor_copy(v4_t[st][:, :, 0:128], ps_v)

                def score_pair(ps0, g, u0, store):
                    """S^T for k-tiles u0,u0+1 of group g -> exp'd fp16 tile."""
                    slots = list(range(4 * g, 4 * g + 4))
                    w0 = grp_w(g, u0)
                    s_sp = ps0.tile([P, 2, 512], f32, tag="s0",
                                    name=f"s_{g}_{u0}")
                    for j in range(2):
                        u = u0 + j
                        wj = grp_w(g, u)
                        nc.tensor.matmul(s_sp[:, j, 0:wj * P],
                                         kt_t[u // 4][:, ts(u % 4, P)],
                                         qt_t[g][:, 0:wj * P],
                                         start=True, stop=True)
                    if store:
                        p_sb = ppool.tile([P, 2, 512], f16, name=f"p_{g}_{u0}")
                        p_st[(g, u0)] = p_sb
                    else:
                        p_sb = wpool.tile([P, 2, 512], f16, tag="ptile0",
                                          name=f"p_{g}_{u0}")
                    nc.scalar.activation(p_sb[:, :, 0:w0 * P],
                                         s_sp[:, :, 0:w0 * P], Exp)
                    for j in range(2):
                        boundary_masks(
                            lambda ci, j=j: p_sb[:, j, ts(ci, P)],
                            u0 + j, slots, grp_w(g, u0 + j))
                    return p_sb

                # ---- phase 1: projections, group-0 attention, and S+exp
                # prefetch for groups 1-3 interleaved over the k strips ----
                slots0 = [0, 1, 2, 3]
                first0 = {0, 2}
                with tc.tile_pool(name="ps_o0", bufs=1, space="PSUM") as po0, \
                     tc.tile_pool(name="pproj", bufs=2, space="PSUM") as pp, \
                     tc.tile_pool(name="ps_s0", bufs=2, space="PSUM") as ps0:
                    ob0 = {i: po0.tile([P, 2, 129], f32,
                                       name=f"o_bank_0_{i}") for i in range(2)}
                    o_ps0 = {s: ob0[ci // 2][:, ci % 2]
                             for ci, s in enumerate(slots0)}
                    def g0_pairs(stp):
                        for up in (2 * stp, 2 * stp + 1):
                            u0 = 2 * up
                            w = grp_w(0, u0)
                            p_sb = score_pair(ps0, 0, u0, store=False)
                            for j in range(2):
                                u = u0 + j
                                for ci, s in enumerate(slots0[:w]):
                                    nc.tensor.matmul(
                                        o_ps0[s], p_sb[:, j, ts(ci, P)],
                                        v4_t[u // 4][:, u % 4, 0:129],
                                        start=(u == 0 and s in first0),
                                        stop=(u == KC[s] - 1),
                                        skip_group_check=True)
                                    if u == KC[s] - 1:
                                        finalize(s, o_ps0[s])

                    # attention pairs run one strip behind the projections so
                    # the in-order Tensor queue never head-blocks on a strip
                    # (or q-block) whose DMA is still in flight; pairs go
                    # first within the iteration since their data is already
                    # resident while strip st's DMA may still be landing.
                    QP = {0: 0, 2: 1, 4: 2, 5: 3}     # strip -> proj_q(g)
                    for st in range(NSTRIP):
                        if st >= 1:
                            g0_pairs(st - 1)
                        proj_kv(pp, st)
                        if st in QP:
                            proj_q(pp, QP[st])
                        for g, u0 in PREF.get(st, ()):
                            score_pair(ps0, g, u0, store=True)
                    g0_pairs(NSTRIP - 1)

                # ---- phase 2: pure PV sweep over the stored exp'd scores,
                # merged u-major across groups 1-3 so the narrow tails of the
                # groups overlap.  Six independent O banks (PSUM freed by the
                # phase-1 pools) so groups never wait on one another's
                # finalize.  O accumulators pack two slots per PSUM bank,
                # relying on per-element has_written: the bank's first PV
                # (slot A, u=0, start=True) clears the bank; slot B's u=0 PV
                # uses start=False and overwrites its still-unwritten
                # elements.
                with tc.tile_pool(name="ps_o2", bufs=1, space="PSUM") as po2, \
                     tc.tile_pool(name="ps_s2", bufs=1, space="PSUM") as ps2:
                    o_ps = {}
                    first_in_bank = set()
                    for g in range(1, 4):
                        slots = list(range(4 * g, 4 * g + 4))
                        ob = {i: po2.tile([P, 2, 129], f32, name=f"o2_{g}_{i}")
                              for i in range(2)}
                        for ci, s in enumerate(slots):
                            o_ps[s] = ob[ci // 2][:, ci % 2]
                        first_in_bank |= {slots[0], slots[2]}
                    for u0 in range(0, KC[4], 2):
                        # scores not prefetched in phase 1 are computed here,
                        # hidden behind the other groups' PV streams (Scalar
                        # has plenty of slack in this phase).
                        for g in (2, 3):
                            if grp_w(g, u0) and (g, u0) not in p_st:
                                p_st[(g, u0)] = score_pair(ps2, g, u0,
                                                           store=False)
                        for jj in range(2):
                            u = u0 + jj
                            for g in range(1, 4):
                                wj = grp_w(g, u)
                                if wj == 0:
                                    continue
                                slots = list(range(4 * g, 4 * g + 4))
                                p_sb = p_st[(g, u0)]
                                for ci, s in enumerate(slots[:wj]):
                                    nc.tensor.matmul(
                                        o_ps[s], p_sb[:, jj, ts(ci, P)],
                                        v4_t[u // 4][:, u % 4, 0:129],
                                        start=(u == 0 and s in first_in_bank),
                                        stop=(u == KC[s] - 1),
                                        skip_group_check=True)
                                    if u == KC[s] - 1:
                                        finalize(s, o_ps[s])

    nc.compile()
    return nc


def _prep_core(core, x, wq, bq, wk, bk, wv, bv):
    b, ct = core // 2, core % 2
    qtiles = _slot_qtile(ct)
    scale = np.float32(1.0 / np.sqrt(H))

    def dchunk(a):  # [D, N] -> [P, DO, N] with d = o*P + p
        return np.ascontiguousarray(
            a.reshape(DO, P, -1).transpose(1, 0, 2)).astype(_f16)

    xT = x[b].T.astype(np.float32)                      # [D, T]
    qrows = np.concatenate([np.arange(j * P, (j + 1) * P) for j in qtiles])
    xqT = np.ascontiguousarray(xT[:, qrows])            # [D, TQ]

    wall = np.stack([dchunk(wq * scale), dchunk(wk), dchunk(wv)], axis=1)
    ball = np.stack([(bq * scale).astype(np.float32),
                     bk.astype(np.float32)], axis=1)

    # boundary masks: per slot, blocks for u = KC-2 and KC-1 in S^T [k, q]
    # orientation: ones below the causal boundary, triu on the diagonal tile,
    # zeros for the padded tile beyond it.
    triu = np.triu(np.ones((P, P), dtype=_f16))
    msk = np.zeros((P, NS * 2 * P), dtype=_f16)
    for s in range(NS):
        j = qtiles[s]
        for i in range(2):
            u = KC[s] - 2 + i
            blk = 2 * s + i
            if u < j:
                msk[:, blk * P:(blk + 1) * P] = 1.0
            elif u == j:
                msk[:, blk * P:(blk + 1) * P] = triu

    return {
        "xT": dchunk(xT),
        "xqT": dchunk(xqT),
        "wall": wall,
        "ball": ball,
        "msk": msk,
    }


def _fallback(x, mask, wq, bq, wk, bk, wv, bv):
    """Exact numpy path for inputs the specialized kernel doesn't cover."""
    out = np.empty((x.shape[0], x.shape[1], wq.shape[1]), dtype=np.float32)
    scale = np.float32(1.0 / np.sqrt(wq.shape[1]))
    for b in range(x.shape[0]):
        q = x[b] @ wq + bq
        k = x[b] @ wk + bk
        v = x[b] @ wv + bv
        s = (q @ k.T) * scale
        s = np.where(mask == 0, np.float32(-1e30), s)
        s -= s.max(axis=-1, keepdims=True)
        p = np.exp(s)
        p /= p.sum(axis=-1, keepdims=True)
        out[b] = p @ v
    return out


def kernel(**inputs):
    x = np.asarray(inputs["x"], dtype=np.float32)
    mask = np.asarray(inputs["mask"])
    wq = np.asarray(inputs["wq"], dtype=np.float32)
    bq = np.asarray(inputs["bq"], dtype=np.float32)
    wk = np.asarray(inputs["wk"], dtype=np.float32)
    bk = np.asarray(inputs["bk"], dtype=np.float32)
    wv = np.asarray(inputs["wv"], dtype=np.float32)
    bv = np.asarray(inputs["bv"], dtype=np.float32)

    causal = (x.shape == (B, T, D) and wq.shape == (D, H)
              and np.array_equal(mask, np.tril(np.ones((T, T), mask.dtype))))
    if not causal:
        return _fallback(x, mask, wq, bq, wk, bk, wv, bv)

    if "nc" not in _CACHE:
        _CACHE["nc"] = _build_program()
    nc = _CACHE["nc"]

    from concourse import bass_utils
    in_maps = [_prep_core(c, x, wq, bq, wk, bk, wv, bv) for c in range(8)]
    res = bass_utils.run_bass_kernel_spmd(nc, in_maps, core_ids=list(range(8)),
                                          **_CACHE.get("run_kwargs", {}))
    _CACHE["last_result"] = res

    out = np.empty((B, T, H), dtype=np.float32)
    bvf = bv.astype(np.float32)
    for c in range(8):
        b, ct = c // 2, c % 2
        qtiles = _slot_qtile(ct)
        oc = res.results[c]["out"]          # [NS, P, 129]
        for s, j in enumerate(qtiles):
            out[b, j * P:(j + 1) * P, :] = (
                oc[s, :, 0:128] / oc[s, :, 128:129] + bvf)
    return out
